# revision 3
# baseline (speedup 1.0000x reference)
"""3x3 neighborhood (ADDA) attention on Trainium2, B=8, d=512 (8 heads x 64), 56x56.

Sharding: pure data parallel per the hint — batch b -> NeuronCore b (8 cores,
SPMD, no cross-core communication). Each core computes full local attention for
one batch.

Device layout (per core, per head-pair group g of 4):
  SBUF tiles [116 partitions = 2 heads x 58 padded rows, free = [64 ch, 60 x]].
  Compute ops must start at partition 0 (HW rule: >64-partition accesses need
  base 0), so the 3x3 window's row shift dy is baked in at DMA time: k and v
  are each loaded in 6 variants (dy in {-1,0,1} x two x-offset parities so odd
  dx reads stay 4B-aligned and the DVE keeps 2x bf16 mode). Pads are zeroed ->
  torch-Unfold zero-pad semantics (OOB window slots contribute logit 0 and
  value 0; softmax runs over all 9 slots).
  QK: 9 DVE tensor_tensor mults + strided channel tensor_reduce. Softmax
  without max subtraction (logits are O(5); exp safe in f32), 1/sqrt(64) scale
  folded into the ACT exp. AV: 9 broadcast mults on DVE, f32 accumulation on
  GPSIMD.

Host side: cast f32->bf16 and transpose [c,y,x]->[y,c,x] so every DMA row is
contiguous; inverse transform + f32 cast on the way out.
"""
import sys

sys.path.insert(0, "/opt/trn_rl_repo")

from contextlib import ExitStack

import ml_dtypes
import numpy as np

import concourse.bacc as bacc
import concourse.tile as tile
from concourse import mybir
from concourse.bass_utils import run_bass_kernel_spmd

B, D, H, W = 8, 512, 56, 56
NH, HD = 8, 64
SCALE = HD ** (-0.5)
N_CORES = 8
NG = 4          # head-pair groups
P58 = 58        # padded rows per head (1 + 56 + 1)
NP = 116        # partitions used (2 heads x 58)
XT = 60         # padded x extent (even, so interiors stay 4B-aligned in bf16)
XI = 2          # interior x start
BF16 = mybir.dt.bfloat16
F32 = mybir.dt.float32

# window offsets; xp is the x-parity variant that keeps the read 4B-aligned
OFFS = [(dy, dx, dx & 1) for dy in (-1, 0, 1) for dx in (-1, 0, 1)]
VARIANTS = sorted({(dy, xp) for dy, dx, xp in OFFS})  # 6 per tensor

_NC_CACHE = {}


def _load_variants(nc, pool, dram, g, zero_pads, tag):
    """DMA one head-pair of `dram` into 6 (dy, xp) shifted SBUF variants."""
    tiles = {}
    for dy, xp in VARIANTS:
        t = pool.tile([NP, HD, XT], BF16, tag=f"{tag}{dy}{xp}")
        if zero_pads:
            nc.gpsimd.memset(t[:], 0.0)
        for hh in range(2):
            head = 2 * g + hh
            p0 = hh * P58 + 1 - dy
            nc.sync.dma_start(
                out=t[p0:p0 + H, :, XI + xp:XI + xp + W],
                in_=dram[0:H, head * HD:(head + 1) * HD, 0:W],
            )
        tiles[(dy, xp)] = t
    return tiles


def _build_program():
    nc = bacc.Bacc("TRN2", target_bir_lowering=False, debug=False,
                   num_devices=N_CORES)
    q_d = nc.declare_dram_parameter("q", [H, D, W], BF16, isOutput=False)
    k_d = nc.declare_dram_parameter("k", [H, D, W], BF16, isOutput=False)
    v_d = nc.declare_dram_parameter("v", [H, D, W], BF16, isOutput=False)
    o_d = nc.declare_dram_parameter("out", [H, D, W], BF16, isOutput=True)

    with tile.TileContext(nc) as tc:
        with ExitStack() as ctx:
            io_pool = ctx.enter_context(tc.tile_pool(name="io", bufs=2))
            tmp_pool = ctx.enter_context(tc.tile_pool(name="tmp", bufs=2))
            sm_pool = ctx.enter_context(tc.tile_pool(name="sm", bufs=2))
            acc_pool = ctx.enter_context(tc.tile_pool(name="acc", bufs=1))

            for g in range(NG):
                # k and v share the same slot family (tag "kv*"): k occupies
                # one buffer during QK, v the other during AV. Pad zeroing is
                # done on the first two allocations of each slot (g < 1 covers
                # both buffers: k -> slot A, v -> slot B); later allocations
                # reuse identical geometry so pads stay zero.
                zero = g < 1
                qt = io_pool.tile([NP, HD, XT], BF16, tag="qt")
                for hh in range(2):
                    head = 2 * g + hh
                    nc.sync.dma_start(
                        out=qt[hh * P58 + 1:hh * P58 + 1 + H, :, XI:XI + W],
                        in_=q_d[0:H, head * HD:(head + 1) * HD, 0:W],
                    )
                kvar = _load_variants(nc, io_pool, k_d, g, zero, "kv")

                L = sm_pool.tile([NP, 9, W], F32, tag="L")
                Pt = sm_pool.tile([NP, 9, W], BF16, tag="P")
                Wt = sm_pool.tile([NP, 9, W], BF16, tag="W")
                S = sm_pool.tile([NP, W], F32, tag="S")
                R = sm_pool.tile([NP, W], F32, tag="R")

                # --- QK ---
                for j, (dy, dx, xp) in enumerate(OFFS):
                    tm = tmp_pool.tile([NP, HD, W], BF16, tag="tm")
                    xb = XI + xp + dx
                    nc.vector.tensor_mul(
                        tm[:, :, :],
                        qt[:, :, XI:XI + W],
                        kvar[(dy, xp)][:, :, xb:xb + W],
                    )
                    nc.vector.tensor_reduce(
                        out=L[:, j, :],
                        in_=tm[:, :, :].transpose([0, 2, 1]),
                        axis=mybir.AxisListType.X,
                        op=mybir.AluOpType.add,
                    )

                # --- softmax (no max subtraction; SCALE folded into exp) ---
                nc.scalar.activation(
                    out=Pt[:, :, :], in_=L[:, :, :],
                    func=mybir.ActivationFunctionType.Exp, scale=float(SCALE),
                )
                nc.vector.tensor_reduce(
                    out=S[:, :],
                    in_=Pt[:, :, :].transpose([0, 2, 1]),
                    axis=mybir.AxisListType.X,
                    op=mybir.AluOpType.add,
                )
                nc.vector.reciprocal(out=R[:, :], in_=S[:, :])
                nc.vector.tensor_mul(
                    Wt[:, :, :],
                    Pt[:, :, :],
                    R[:, :].unsqueeze(1).to_broadcast((NP, 9, W)),
                )

                # --- AV ---
                vvar = _load_variants(nc, io_pool, v_d, g, zero, "kv")
                av = acc_pool.tile([NP, HD, W], F32, tag="av")
                for j, (dy, dx, xp) in enumerate(OFFS):
                    ta = tmp_pool.tile([NP, HD, W], BF16, tag="ta")
                    xb = XI + xp + dx
                    nc.vector.tensor_mul(
                        ta[:, :, :],
                        Wt[:, j:j + 1, :].to_broadcast((NP, HD, W)),
                        vvar[(dy, xp)][:, :, xb:xb + W],
                    )
                    if j == 0:
                        nc.gpsimd.tensor_copy(out=av[:, :, :], in_=ta[:, :, :])
                    else:
                        nc.gpsimd.tensor_add(av[:, :, :], av[:, :, :],
                                             ta[:, :, :])

                ob = acc_pool.tile([NP, HD, W], BF16, tag="ob")
                nc.scalar.copy(ob[:, :, :], av[:, :, :])

                for hh in range(2):
                    head = 2 * g + hh
                    nc.sync.dma_start(
                        out=o_d[0:H, head * HD:(head + 1) * HD, 0:W],
                        in_=ob[hh * P58 + 1:hh * P58 + 1 + H, :, :],
                    )

    nc.compile()
    return nc


def _get_nc():
    if "nc" not in _NC_CACHE:
        _NC_CACHE["nc"] = _build_program()
    return _NC_CACHE["nc"]


def _run(q, k, v, trace=False, tmpdir=None):
    q = np.asarray(q, dtype=np.float32)
    k = np.asarray(k, dtype=np.float32)
    v = np.asarray(v, dtype=np.float32)
    # [B, c, y, x] -> [B, y, c, x], bf16
    qh = np.ascontiguousarray(q.transpose(0, 2, 1, 3)).astype(ml_dtypes.bfloat16)
    kh = np.ascontiguousarray(k.transpose(0, 2, 1, 3)).astype(ml_dtypes.bfloat16)
    vh = np.ascontiguousarray(v.transpose(0, 2, 1, 3)).astype(ml_dtypes.bfloat16)
    in_maps = [{"q": qh[b], "k": kh[b], "v": vh[b]} for b in range(N_CORES)]
    nc = _get_nc()
    res = run_bass_kernel_spmd(nc, in_maps, core_ids=list(range(N_CORES)),
                               trace=trace, tmpdir=tmpdir)
    outs = np.stack([np.asarray(res.results[b]["out"]) for b in range(N_CORES)])
    # [B, y, c, x] bf16 -> [B, y, x, c] f32
    out = outs.astype(np.float32).transpose(0, 1, 3, 2)
    return np.ascontiguousarray(out), res


def kernel(q, k, v):
    out, _ = _run(q, k, v, trace=False)
    return out


def run_traced(q, k, v, tmpdir=None):
    out, res = _run(q, k, v, trace=True, tmpdir=tmpdir)
    return out, res


# revision 5
# speedup vs baseline: 1.2112x; 1.2112x over previous
"""3x3 neighborhood (ADDA) attention on Trainium2, B=8, d=512 (8 heads x 64), 56x56.

Sharding: pure data parallel per the hint — batch b -> NeuronCore b (8 cores,
SPMD, no cross-core communication). Each core computes full local attention for
one batch.

Device design (per core, 4 head-pair groups):
  All compute ops run on partitions [0:116) = 2 heads x 58 padded rows (HW
  requires >64-partition accesses to start at partition 0, so the window's row
  shift dy is baked in at DMA time from host-prepared zero-padded images; OOB
  window slots contribute logit 0 / value 0, matching torch-Unfold zero-pad
  semantics, and softmax runs over all 9 slots).

  QK stage uses channel-INNER tiles [116, 60x, 64c]: 9 DVE bf16 mults (2x
  mode; dx shifts are 64-element offsets, always 4B-aligned) + contiguous
  channel tensor_reduce into L[:, j, :]. Softmax without max subtraction
  (logits are O(5)) with the 1/sqrt(64) scale folded into the ACT exp.
  AV stage uses channel-OUTER tiles [116, 64c, 60x] so the per-pixel weight
  broadcasts along the outer free dim and mults stay in 2x mode (v comes in
  two x-parity variants to keep odd dx aligned). The 9 AV products are summed
  on the otherwise idle TensorEngine: identity-weight matmuls accumulating
  into PSUM f32. ACT casts PSUM -> bf16 output tile.

Host side: builds the exact padded SBUF images (bf16) so every DMA is a
contiguous 7.5KB-per-partition run; inverse transform + f32 cast on output.
"""
import sys

sys.path.insert(0, "/opt/trn_rl_repo")

from contextlib import ExitStack

import ml_dtypes
import numpy as np

import concourse.bacc as bacc
import concourse.tile as tile
from concourse import mybir
from concourse.bass_utils import run_bass_kernel_spmd

B, D, H, W = 8, 512, 56, 56
NH, HD = 8, 64
SCALE = HD ** (-0.5)
N_CORES = 8
NG = 4          # head-pair groups
P58 = 58        # tile rows per head (1 + 56 + 1)
NP = 116        # compute partitions (2 heads x 58)
R60 = 60        # image rows per head (2 + 56 + 2) for dy-shifted loads
XT = 60         # padded x extent (even -> interiors stay 4B-aligned in bf16)
XI = 2          # interior x start
FLAT = HD * W   # 3584
BF16 = mybir.dt.bfloat16
F32 = mybir.dt.float32
BF = ml_dtypes.bfloat16

OFFS = [(dy, dx, dx & 1) for dy in (-1, 0, 1) for dx in (-1, 0, 1)]

_NC_CACHE = {}


def _build_program():
    nc = bacc.Bacc("TRN2", target_bir_lowering=False, debug=False,
                   num_devices=N_CORES)
    # host-prepared SBUF images (see _prep_inputs for layouts)
    q_d = nc.declare_dram_parameter("q", [NG, NP, XT, HD], BF16, isOutput=False)
    k_d = nc.declare_dram_parameter("k", [NG, 2 * R60, XT, HD], BF16,
                                    isOutput=False)
    v0_d = nc.declare_dram_parameter("v0", [NG, 2 * R60, HD, XT], BF16,
                                     isOutput=False)
    v1_d = nc.declare_dram_parameter("v1", [NG, 2 * R60, HD, XT], BF16,
                                     isOutput=False)
    i_d = nc.declare_dram_parameter("ident", [NP, NP], BF16, isOutput=False)
    o_d = nc.declare_dram_parameter("out", [NG, NP, HD, W], BF16, isOutput=True)

    with tile.TileContext(nc) as tc:
        with ExitStack() as ctx:
            one_pool = ctx.enter_context(tc.tile_pool(name="one", bufs=1))
            io_pool = ctx.enter_context(tc.tile_pool(name="io", bufs=2))
            tmp_pool = ctx.enter_context(tc.tile_pool(name="tmp", bufs=2))
            sm_pool = ctx.enter_context(tc.tile_pool(name="sm", bufs=2))
            ob_pool = ctx.enter_context(tc.tile_pool(name="ob", bufs=2))
            ps_pool = ctx.enter_context(
                tc.tile_pool(name="ps", bufs=1, space="PSUM"))

            ident = one_pool.tile([NP, NP], BF16)
            nc.sync.dma_start(out=ident[:], in_=i_d[:])

            for g in range(NG):
                qt = io_pool.tile([NP, XT, HD], BF16, tag="qt")
                nc.sync.dma_start(out=qt[:], in_=q_d[g])

                # k variants: row-shifted loads from the 60-row-per-head image
                kvar = {}
                for dy in (-1, 0, 1):
                    t = io_pool.tile([NP, XT, HD], BF16, tag=f"kv{dy}0")
                    for hh in range(2):
                        nc.sync.dma_start(
                            out=t[hh * P58:(hh + 1) * P58],
                            in_=k_d[g, hh * R60 + 1 + dy:
                                    hh * R60 + 1 + dy + P58],
                        )
                    kvar[dy] = t

                L = sm_pool.tile([NP, 9, W], F32, tag="L")
                Pt = sm_pool.tile([NP, 9, W], BF16, tag="P")
                Wt = sm_pool.tile([NP, 9, W], BF16, tag="W")
                S = sm_pool.tile([NP, W], F32, tag="S")
                R = sm_pool.tile([NP, W], F32, tag="R")

                # --- QK: logits (channel-inner) ---
                for j, (dy, dx, xp) in enumerate(OFFS):
                    tm = tmp_pool.tile([NP, W, HD], BF16, tag="tm")
                    nc.vector.tensor_mul(
                        tm[:, :, :],
                        qt[:, XI:XI + W, :],
                        kvar[dy][:, XI + dx:XI + dx + W, :],
                    )
                    nc.vector.tensor_reduce(
                        out=L[:, j, :],
                        in_=tm[:, :, :],
                        axis=mybir.AxisListType.X,
                        op=mybir.AluOpType.add,
                    )

                # --- softmax (no max subtraction; SCALE folded into exp) ---
                nc.scalar.activation(
                    out=Pt[:, :, :], in_=L[:, :, :],
                    func=mybir.ActivationFunctionType.Exp, scale=float(SCALE),
                )
                nc.vector.tensor_reduce(
                    out=S[:, :],
                    in_=Pt[:, :, :].transpose([0, 2, 1]),
                    axis=mybir.AxisListType.X,
                    op=mybir.AluOpType.add,
                )
                nc.vector.reciprocal(out=R[:, :], in_=S[:, :])
                nc.vector.tensor_mul(
                    Wt[:, :, :],
                    Pt[:, :, :],
                    R[:, :].unsqueeze(1).to_broadcast((NP, 9, W)),
                )

                # --- AV: products (channel-outer) + PE identity-accumulate ---
                vvar = {}
                for dy in (-1, 0, 1):
                    for xp, vd in ((0, v0_d), (1, v1_d)):
                        t = io_pool.tile([NP, HD, XT], BF16, tag=f"kv{dy}{xp}")
                        for hh in range(2):
                            nc.sync.dma_start(
                                out=t[hh * P58:(hh + 1) * P58],
                                in_=vd[g, hh * R60 + 1 + dy:
                                       hh * R60 + 1 + dy + P58],
                            )
                        vvar[(dy, xp)] = t

                av = ps_pool.tile([NP, FLAT], F32, tag="av")
                for j, (dy, dx, xp) in enumerate(OFFS):
                    ta = tmp_pool.tile([NP, HD, W], BF16, tag="ta")
                    xb = XI + xp + dx
                    nc.vector.tensor_mul(
                        ta[:, :, :],
                        Wt[:, j:j + 1, :].to_broadcast((NP, HD, W)),
                        vvar[(dy, xp)][:, :, xb:xb + W],
                    )
                    taf = ta[:, :, :].rearrange("p c x -> p (c x)")
                    for ch in range(FLAT // 512):
                        nc.tensor.matmul(
                            av[:, ch * 512:(ch + 1) * 512],
                            ident[:],
                            taf[:, ch * 512:(ch + 1) * 512],
                            start=(j == 0),
                            stop=(j == 8),
                        )

                ob = ob_pool.tile([NP, HD, W], BF16, tag="ob")
                nc.scalar.copy(ob[:, :, :], av[:, :].rearrange(
                    "p (c x) -> p c x", c=HD))
                nc.sync.dma_start(out=o_d[g], in_=ob[:])

    nc.compile()
    return nc


def _get_nc():
    if "nc" not in _NC_CACHE:
        _NC_CACHE["nc"] = _build_program()
    return _NC_CACHE["nc"]


def _prep_inputs(q, k, v):
    """Build per-core host images.

    q image  [NG, 116, 60, 64]  (row-padded, x-padded, channel-inner)
    k image  [NG, 120, 60, 64]  (60 rows/head: 2+56+2 zero-guarded, ch-inner)
    v images [NG, 120, 64, 60]  x-parity 0/1 (channel-outer)
    """
    in_maps = []
    ident = np.eye(NP, dtype=BF)
    for b in range(N_CORES):
        # [head, c, y, x] -> per-head [y, x, c] and [y, c, x]
        qb = q[b].reshape(NH, HD, H, W)
        kb = k[b].reshape(NH, HD, H, W)
        vb = v[b].reshape(NH, HD, H, W)
        qyxc = qb.transpose(0, 2, 3, 1).astype(BF)   # [h, y, x, c]
        kyxc = kb.transpose(0, 2, 3, 1).astype(BF)
        vycx = vb.transpose(0, 2, 1, 3).astype(BF)   # [h, y, c, x]

        qi = np.zeros((NG, NP, XT, HD), dtype=BF)
        ki = np.zeros((NG, 2 * R60, XT, HD), dtype=BF)
        v0i = np.zeros((NG, 2 * R60, HD, XT), dtype=BF)
        v1i = np.zeros((NG, 2 * R60, HD, XT), dtype=BF)
        for g in range(NG):
            for hh in range(2):
                hd = 2 * g + hh
                qi[g, hh * P58 + 1:hh * P58 + 1 + H, XI:XI + W, :] = qyxc[hd]
                ki[g, hh * R60 + 2:hh * R60 + 2 + H, XI:XI + W, :] = kyxc[hd]
                v0i[g, hh * R60 + 2:hh * R60 + 2 + H, :, XI:XI + W] = vycx[hd]
                v1i[g, hh * R60 + 2:hh * R60 + 2 + H, :, XI + 1:XI + 1 + W] = \
                    vycx[hd]
        in_maps.append({"q": qi, "k": ki, "v0": v0i, "v1": v1i,
                        "ident": ident})
    return in_maps


def _run(q, k, v, trace=False, tmpdir=None):
    q = np.asarray(q, dtype=np.float32)
    k = np.asarray(k, dtype=np.float32)
    v = np.asarray(v, dtype=np.float32)
    in_maps = _prep_inputs(q, k, v)
    nc = _get_nc()
    res = run_bass_kernel_spmd(nc, in_maps, core_ids=list(range(N_CORES)),
                               trace=trace, tmpdir=tmpdir)
    # out image [NG, 116, 64, 56] -> [y, x, c]
    out = np.empty((B, H, W, D), dtype=np.float32)
    for b in range(N_CORES):
        oi = np.asarray(res.results[b]["out"]).astype(np.float32)
        for g in range(NG):
            for hh in range(2):
                hd = 2 * g + hh
                blk = oi[g, hh * P58 + 1:hh * P58 + 1 + H]     # [y, c, x]
                out[b, :, :, hd * HD:(hd + 1) * HD] = blk.transpose(0, 2, 1)
    return out, res


def kernel(q, k, v):
    out, _ = _run(q, k, v, trace=False)
    return out


def run_traced(q, k, v, tmpdir=None):
    out, res = _run(q, k, v, trace=True, tmpdir=tmpdir)
    return out, res


# revision 6
# speedup vs baseline: 1.8823x; 1.5541x over previous
"""3x3 neighborhood (ADDA) attention on Trainium2, B=8, d=512 (8 heads x 64), 56x56.

Sharding: pure data parallel per the hint — batch b -> NeuronCore b (8 cores,
SPMD, no cross-core communication). Each core computes full local attention for
one batch.

Device design (per core, 4 head-pair groups):
  All compute ops run on partitions [0:116) = 2 heads x 58 padded rows (HW
  requires >64-partition accesses to start at partition 0, so the window's row
  shift dy is baked into host-prepared zero-padded images; OOB window slots
  contribute logit 0 / value 0, matching torch-Unfold zero-pad semantics, and
  softmax runs over all 9 slots).

  QK stage uses channel-INNER tiles [116, 60x, 64c]: 9 DVE bf16 mults (2x
  mode) + contiguous channel tensor_reduce into L[:, j, :]. Softmax without
  max subtraction (logits are O(5)) with the 1/sqrt(64) scale folded into the
  ACT exp. AV stage uses channel-OUTER tiles [116, 64c, 60x] so the per-pixel
  weight broadcasts along the outer free dim and mults stay in 2x mode (v has
  two x-parity variants to keep odd dx 4B-aligned). The 9 AV products are
  summed on the otherwise idle TensorEngine: identity-weight matmuls
  accumulating into PSUM f32; ACT casts PSUM -> bf16 output.

  DMA: descriptor GENERATION on the issuing sequencer is the scaling limit
  (~one descriptor per partition row per DMA), so each tensor family arrives
  as ONE DMA of a host-concatenated variant image (k: 3 dy variants, v: 6
  dy/x-parity variants, contiguous per partition), issued on the GPSIMD SWDGE
  (parallel descriptor generation) while the output store uses the SP HWDGE.
"""
import sys

sys.path.insert(0, "/opt/trn_rl_repo")

from contextlib import ExitStack

import ml_dtypes
import numpy as np

import concourse.bacc as bacc
import concourse.tile as tile
from concourse import mybir
from concourse.bass_utils import run_bass_kernel_spmd

B, D, H, W = 8, 512, 56, 56
NH, HD = 8, 64
SCALE = HD ** (-0.5)
N_CORES = 8
NG = 4          # head-pair groups
P58 = 58        # tile rows per head (1 + 56 + 1)
NP = 116        # compute partitions (2 heads x 58)
XT = 60         # padded x extent (even -> interiors stay 4B-aligned in bf16)
XI = 2          # interior x start
FLAT = HD * W   # 3584
BF16 = mybir.dt.bfloat16
F32 = mybir.dt.float32
BF = ml_dtypes.bfloat16

OFFS = [(dy, dx, dx & 1) for dy in (-1, 0, 1) for dx in (-1, 0, 1)]
VVARS = [(dy, xp) for dy in (-1, 0, 1) for xp in (0, 1)]

_NC_CACHE = {}


def _build_program():
    nc = bacc.Bacc("TRN2", target_bir_lowering=False, debug=False,
                   num_devices=N_CORES)
    q_d = nc.declare_dram_parameter("q", [NG, NP, XT, HD], BF16, isOutput=False)
    k_d = nc.declare_dram_parameter("k", [NG, NP, 3, XT, HD], BF16,
                                    isOutput=False)
    v_d = nc.declare_dram_parameter("v", [NG, NP, 6, HD, XT], BF16,
                                    isOutput=False)
    i_d = nc.declare_dram_parameter("ident", [NP, NP], BF16, isOutput=False)
    o_d = nc.declare_dram_parameter("out", [NG, NP, HD, W], BF16, isOutput=True)

    with tile.TileContext(nc) as tc:
        with ExitStack() as ctx:
            one_pool = ctx.enter_context(tc.tile_pool(name="one", bufs=1))
            io_pool = ctx.enter_context(tc.tile_pool(name="io", bufs=2))
            tmp_pool = ctx.enter_context(tc.tile_pool(name="tmp", bufs=2))
            sm_pool = ctx.enter_context(tc.tile_pool(name="sm", bufs=2))
            ob_pool = ctx.enter_context(tc.tile_pool(name="ob", bufs=2))
            ps_pool = ctx.enter_context(
                tc.tile_pool(name="ps", bufs=1, space="PSUM"))

            ident = one_pool.tile([NP, NP], BF16)
            nc.sync.dma_start(out=ident[:], in_=i_d[:])

            for g in range(NG):
                qt = io_pool.tile([NP, XT, HD], BF16, tag="qt")
                nc.gpsimd.dma_start(out=qt[:], in_=q_d[g])
                # k family: 3 dy-shifted variant images in one DMA
                ka = io_pool.tile([NP, 3, XT, HD], BF16, tag="kv")
                nc.gpsimd.dma_start(out=ka[:], in_=k_d[g])

                L = sm_pool.tile([NP, 9, W], F32, tag="L")
                Pt = sm_pool.tile([NP, 9, W], BF16, tag="P")
                Wt = sm_pool.tile([NP, 9, W], BF16, tag="W")
                S = sm_pool.tile([NP, W], F32, tag="S")
                R = sm_pool.tile([NP, W], F32, tag="R")

                # --- QK: logits (channel-inner) ---
                for j, (dy, dx, xp) in enumerate(OFFS):
                    tm = tmp_pool.tile([NP, W, HD], BF16, tag="tm")
                    nc.vector.tensor_mul(
                        tm[:, :, :],
                        qt[:, XI:XI + W, :],
                        ka[:, dy + 1, XI + dx:XI + dx + W, :],
                    )
                    nc.vector.tensor_reduce(
                        out=L[:, j, :],
                        in_=tm[:, :, :],
                        axis=mybir.AxisListType.X,
                        op=mybir.AluOpType.add,
                    )

                # --- softmax (no max subtraction; SCALE folded into exp) ---
                nc.scalar.activation(
                    out=Pt[:, :, :], in_=L[:, :, :],
                    func=mybir.ActivationFunctionType.Exp, scale=float(SCALE),
                )
                nc.vector.tensor_reduce(
                    out=S[:, :],
                    in_=Pt[:, :, :].transpose([0, 2, 1]),
                    axis=mybir.AxisListType.X,
                    op=mybir.AluOpType.add,
                )
                nc.vector.reciprocal(out=R[:, :], in_=S[:, :])
                nc.vector.tensor_mul(
                    Wt[:, :, :],
                    Pt[:, :, :],
                    R[:, :].unsqueeze(1).to_broadcast((NP, 9, W)),
                )

                # --- AV: products (channel-outer) + PE identity-accumulate ---
                va = io_pool.tile([NP, 6, HD, XT], BF16, tag="kv")
                nc.gpsimd.dma_start(out=va[:], in_=v_d[g])

                av = ps_pool.tile([NP, FLAT], F32, tag="av")
                for j, (dy, dx, xp) in enumerate(OFFS):
                    ta = tmp_pool.tile([NP, HD, W], BF16, tag="ta")
                    xb = XI + xp + dx
                    nc.vector.tensor_mul(
                        ta[:, :, :],
                        Wt[:, j:j + 1, :].to_broadcast((NP, HD, W)),
                        va[:, VVARS.index((dy, xp)), :, xb:xb + W],
                    )
                    taf = ta[:, :, :].rearrange("p c x -> p (c x)")
                    for ch in range(FLAT // 512):
                        nc.tensor.matmul(
                            av[:, ch * 512:(ch + 1) * 512],
                            ident[:],
                            taf[:, ch * 512:(ch + 1) * 512],
                            start=(j == 0),
                            stop=(j == 8),
                        )

                ob = ob_pool.tile([NP, HD, W], BF16, tag="ob")
                nc.scalar.copy(ob[:, :, :], av[:, :].rearrange(
                    "p (c x) -> p c x", c=HD))
                nc.sync.dma_start(out=o_d[g], in_=ob[:])

    nc.compile()
    return nc


def _get_nc():
    if "nc" not in _NC_CACHE:
        _NC_CACHE["nc"] = _build_program()
    return _NC_CACHE["nc"]


def _prep_inputs(q, k, v):
    """Build per-core variant images (leading dim = core/batch).

    q: [B, NG, 116, 60, 64]    k: [B, NG, 116, 3, 60, 64] (dy in {-1,0,1})
    v: [B, NG, 116, 6, 64, 60] ((dy, xp) in VVARS order)
    Tile row p = hh*58 + pr holds image row y = pr - 1 (+dy for variants);
    out-of-range rows and x pads are zero.
    """
    qyxc = q.reshape(B, NH, HD, H, W).transpose(0, 1, 3, 4, 2).astype(BF)
    kyxc = k.reshape(B, NH, HD, H, W).transpose(0, 1, 3, 4, 2).astype(BF)
    vycx = v.reshape(B, NH, HD, H, W).transpose(0, 1, 3, 2, 4).astype(BF)

    qi = np.zeros((B, NG, NP, XT, HD), dtype=BF)
    ki = np.zeros((B, NG, NP, 3, XT, HD), dtype=BF)
    vi = np.zeros((B, NG, NP, 6, HD, XT), dtype=BF)
    for g in range(NG):
        for hh in range(2):
            hd = 2 * g + hh
            p0 = hh * P58
            qi[:, g, p0 + 1:p0 + 1 + H, XI:XI + W, :] = qyxc[:, hd]
            for di, dy in enumerate((-1, 0, 1)):
                a, b = max(0, 1 - dy), min(P58, P58 - 1 - dy)
                ki[:, g, p0 + a:p0 + b, di, XI:XI + W, :] = \
                    kyxc[:, hd, a - 1 + dy:b - 1 + dy]
                for xp in (0, 1):
                    vi[:, g, p0 + a:p0 + b, VVARS.index((dy, xp)), :,
                       XI + xp:XI + xp + W] = vycx[:, hd, a - 1 + dy:b - 1 + dy]
    ident = np.eye(NP, dtype=BF)
    return [{"q": qi[b], "k": ki[b], "v": vi[b], "ident": ident}
            for b in range(N_CORES)]


def _run(q, k, v, trace=False, tmpdir=None):
    q = np.asarray(q, dtype=np.float32)
    k = np.asarray(k, dtype=np.float32)
    v = np.asarray(v, dtype=np.float32)
    in_maps = _prep_inputs(q, k, v)
    nc = _get_nc()
    res = run_bass_kernel_spmd(nc, in_maps, core_ids=list(range(N_CORES)),
                               trace=trace, tmpdir=tmpdir)
    # out image [NG, 116, 64, 56] -> [y, x, c]
    out = np.empty((B, H, W, D), dtype=np.float32)
    for b in range(N_CORES):
        oi = np.asarray(res.results[b]["out"]).astype(np.float32)
        for g in range(NG):
            for hh in range(2):
                hd = 2 * g + hh
                blk = oi[g, hh * P58 + 1:hh * P58 + 1 + H]     # [y, c, x]
                out[b, :, :, hd * HD:(hd + 1) * HD] = blk.transpose(0, 2, 1)
    return out, res


def kernel(q, k, v):
    out, _ = _run(q, k, v, trace=False)
    return out


def run_traced(q, k, v, tmpdir=None):
    out, res = _run(q, k, v, trace=True, tmpdir=tmpdir)
    return out, res
